# revision 14
# baseline (speedup 1.0000x reference)
"""Trainium2 Bass kernel for nn_MultiHeadCrossGraphAttention.

Row-parallel attention: the query dimension (Nq=4096) is sharded across 8
NeuronCores (512 queries each); every core holds all keys/values/positions.
Outputs: (out [Nq, 64], attn [8, Nq, 2048]) — matching the reference tuple.

Self-contained: hardcodes shapes/sharding, imports only concourse from the
container install.
"""

import math
import os
import sys

import numpy as np

for _p in ("/opt/trn_rl_repo",):
    if _p not in sys.path and os.path.isdir(_p):
        sys.path.insert(0, _p)

import ml_dtypes  # noqa: E402

N_CORES = 8
NQ_FULL = 4096
NK_FULL = 2048
D_MODEL = 64
N_HEADS = 8
D_K = 8
LN_EPS = 1e-5

# ---------------------------------------------------------------------------
# Device program
# ---------------------------------------------------------------------------

_PROG_CACHE = {}


def build_program(nql=NQ_FULL // N_CORES, nk=NK_FULL, norm_engine="gpsimd",
                  copy_split=20):
    """Builds the per-core Bass program (SPMD: same program on all cores).

    nql: queries per core (multiple of 128). nk: keys (multiple of 1024).
    norm_engine: "gpsimd" or "vector" for the attn normalize pass.
    copy_split: out of the P^T PSUM->SBUF copies, every copy_split-th goes to
        ACT instead of DVE (0 = all DVE).
    """
    import concourse.bass as bass
    import concourse.tile as tile
    from concourse import bacc, mybir

    f32 = mybir.dt.float32
    f32r = mybir.dt.float32r
    bf16 = mybir.dt.bfloat16

    nqt = nql // 128          # query tiles of 128
    nkc = nk // 512           # 512-wide key chunks
    nkh = nk // 1024          # 1024-wide key halves (psum granularity)
    nk128 = nk // 128         # 128-wide key chunks (transpose granularity)

    nc = bacc.Bacc("TRN2", target_bir_lowering=False, debug=False,
                   num_devices=N_CORES)

    dram = {}

    def din(name, shape, dt=f32):
        dram[name] = nc.dram_tensor(name, shape, dt, kind="ExternalInput").ap()

    def dout(name, shape, dt=f32):
        dram[name] = nc.dram_tensor(name, shape, dt, kind="ExternalOutput").ap()

    nb65 = nql + 2 * nk + 4 * 128 + D_MODEL
    din("blob65", [D_MODEL + 1, nb65])
    din("blob5", [5, nql + nk])
    din("blob128", [128, 2 + 2 * D_MODEL])
    din("wo_stk", [D_K, N_HEADS * D_MODEL])
    din("qfres128", [128, (nql // 128) * D_MODEL])
    din("ident_bf16", [128, 128], bf16)

    dout("attn_o", [N_HEADS, nql, nk])
    dout("out_o", [nql, D_MODEL])

    with tile.TileContext(nc) as tc:
        _emit(nc, tc, dram, nql, nk, nqt, nkc, nkh, nk128,
              f32, f32r, bf16, mybir, norm_engine, copy_split)

    nc.compile()
    return nc


def _emit(nc, tc, dram, nql, nk, nqt, nkc, nkh, nk128,
          f32, f32r, bf16, mybir, norm_engine, copy_split):
    from contextlib import ExitStack

    AF = mybir.ActivationFunctionType
    OP = mybir.AluOpType

    with ExitStack() as ctx:
        const = ctx.enter_context(tc.tile_pool(name="const", bufs=1))

        def load_const(name, shape, dt=f32):
            t = const.tile(shape, dt, tag=name, name=name)
            nc.sync.dma_start(out=t, in_=dram[name])
            return t

        nb65 = nql + 2 * nk + 4 * 128 + D_MODEL
        blob65 = load_const("blob65", [D_MODEL + 1, nb65])
        blob5 = load_const("blob5", [5, nql + nk])
        blob128 = load_const("blob128", [128, 2 + 2 * D_MODEL])
        wo_stk = load_const("wo_stk", [D_K, N_HEADS * D_MODEL])
        qfres128 = load_const("qfres128", [128, nqt * D_MODEL])
        ident_b = load_const("ident_bf16", [128, 128], bf16)

        o = 0
        qfT = blob65[:, o:o + nql]; o += nql
        kfT = blob65[:, o:o + nk]; o += nk
        vfT = blob65[:, o:o + nk]; o += nk
        wq_slot = [blob65[:, o:o + 128], blob65[:, o + 128:o + 256]]; o += 256
        wk_slot = [blob65[:, o:o + 128], blob65[:, o + 128:o + 256]]; o += 256
        wv_aug = blob65[:, o:o + D_MODEL]; o += D_MODEL
        qaug = blob5[:, 0:nql]
        kaug = blob5[:, nql:nql + nk]
        sig_scale = blob128[:, 0:1]
        sig_bias = blob128[:, 1:2]
        lng_bc = blob128[:, 2:2 + D_MODEL]
        lnb_bc = blob128[:, 2 + D_MODEL:2 + 2 * D_MODEL]
        ident_f = const.tile([128, 128], f32, tag="ident_f", name="ident_f")
        ident_r = const.tile([128, 128], f32r, tag="ident_r", name="ident_r")
        from concourse.masks import make_identity
        make_identity(nc, ident_f)
        nc.scalar.copy(ident_r, ident_f)

        def pguard(ps_tile, dep=None, cast=True):
            # tiny transpose into a fresh PSUM tile: absorbs the slot-release
            # semaphore wait so the real (hw-decoded) matmul carries <=1 wait
            dst = ps_tile.bitcast(bf16) if cast else ps_tile
            d = ident_b if dep is None else dep
            nc.tensor.transpose(dst[0:1, 0:1], d[0:1, 0:1], ident_b[0:1, 0:1])

        # Projection results (live for the whole kernel)
        qt_sb = [const.tile([128, nql], f32r, tag=f"qt{X}", name=f"qt{X}") for X in range(2)]
        kt_sb = [const.tile([128, nk], f32r, tag=f"kt{X}", name=f"kt{X}") for X in range(2)]
        v_sb = const.tile([128, nk128 * D_MODEL], bf16, tag="v_sb", name="v_sb")
        sp_sb = [const.tile([128, nk], f32r, tag=f"sp{t}", name=f"sp{t}") for t in range(nqt)]
        eps_t = const.tile([128, 1], f32, tag="eps_t", name="eps_t")
        nc.vector.memset(eps_t, LN_EPS)
        # residual accumulators (init = qf + bo), slices of one DMA'd tile
        xacc = [qfres128[:, t * D_MODEL:(t + 1) * D_MODEL] for t in range(nqt)]

        # ------------------------------------------------------------------
        # Stage 1: projections QT (head-packed), KT, V
        # ------------------------------------------------------------------
        cq = min(512, nql)
        with tc.tile_pool(name="ps1", bufs=2, space="PSUM") as ps1:
            for X in range(2):
                for c in range(nql // cq):
                    pq = ps1.tile([128, cq], f32, tag="ps1", name="ps1")
                    nc.tensor.matmul(pq,
                                     lhsT=wq_slot[X],
                                     rhs=qfT[:, c * cq:(c + 1) * cq],
                                     start=True, stop=True)
                    nc.scalar.copy(qt_sb[X][:, c * cq:(c + 1) * cq], pq)
            for X in range(2):
                for c in range(nkc):
                    pk = ps1.tile([128, 512], f32, tag="ps1", name="ps1")
                    nc.tensor.matmul(pk, lhsT=wk_slot[X],
                                     rhs=kfT[:, c * 512:(c + 1) * 512],
                                     start=True, stop=True)
                    nc.scalar.copy(kt_sb[X][:, c * 512:(c + 1) * 512], pk)
            for kc in range(nk128):
                pv = ps1.tile([128, D_MODEL], f32, tag="psv", name="psv")
                nc.tensor.matmul(pv, lhsT=vfT[:, kc * 128:(kc + 1) * 128],
                                 rhs=wv_aug, start=True, stop=True)
                nc.scalar.copy(v_sb[:, kc * D_MODEL:(kc + 1) * D_MODEL], pv)

        # ------------------------------------------------------------------
        # Stage 2: spatial bias  sigmoid(C * sqrt(max(d2, 0)) + bs2)
        # ------------------------------------------------------------------
        with tc.tile_pool(name="ps2", bufs=3, space="PSUM") as ps2:
            for t in range(nqt):
                for c in range(nkc):
                    pd = ps2.tile([128, 512], f32, tag="ps2", name="ps2")
                    nc.tensor.matmul(pd,
                                     lhsT=qaug[:, t * 128:(t + 1) * 128],
                                     rhs=kaug[:, c * 512:(c + 1) * 512],
                                     start=True, stop=True)
                    nc.vector.tensor_scalar_max(
                        sp_sb[t][:, c * 512:(c + 1) * 512], pd, 0.0)
        for t in range(nqt):
            nc.scalar.activation(sp_sb[t], sp_sb[t], AF.Sqrt)
        for t in range(nqt):
            nc.scalar.activation(sp_sb[t], sp_sb[t], AF.Sigmoid,
                                 bias=sig_bias, scale=sig_scale)

        # ------------------------------------------------------------------
        # Stage 3: attention main loop (per head)
        # ------------------------------------------------------------------
        l_pool = ctx.enter_context(
            tc.tile_pool(name="l_ps", bufs=2, space="PSUM"))
        t_pool = ctx.enter_context(
            tc.tile_pool(name="t_ps", bufs=1, space="PSUM"))
        av_pool = ctx.enter_context(
            tc.tile_pool(name="av_ps", bufs=1, space="PSUM"))
        pp_pool = ctx.enter_context(
            tc.tile_pool(name="pp_ps", bufs=1, space="PSUM"))
        p_pool = ctx.enter_context(tc.tile_pool(name="p_sb", bufs=6))
        pt_pool = ctx.enter_context(tc.tile_pool(name="pt_sb", bufs=2))
        a_pool = ctx.enter_context(tc.tile_pool(name="a_sb", bufs=3))
        z_pool = ctx.enter_context(tc.tile_pool(name="z_sb", bufs=16))
        avt_pool = ctx.enter_context(tc.tile_pool(name="avt_sb", bufs=2))

        copy_i = 0
        for h in range(N_HEADS):
            X, jj = h // 4, (h % 4) * 32
            PT = pt_pool.tile([128, nk128, nql], bf16, tag="pt", name="pt")
            rz_t = []
            for t in range(nqt):
                P = p_pool.tile([128, nk], bf16, tag="p", name="p")
                Zc = z_pool.tile([128, nkh], f32, tag="zc", name="zc")
                for half in range(nkh):
                    L = l_pool.tile([128, 1024], f32, tag="l", name="l")
                    for c2 in range(2):
                        k0 = half * 1024 + c2 * 512
                        sl = slice(c2 * 512, (c2 + 1) * 512)
                        nc.tensor.matmul(
                            L[:, sl],
                            lhsT=qt_sb[X][jj:jj + D_K,
                                          t * 128:(t + 1) * 128],
                            rhs=kt_sb[X][jj:jj + D_K,
                                         k0:k0 + 512],
                            start=True, stop=False,
                            tile_position=(jj, 0))
                        nc.tensor.matmul(
                            L[:, sl],
                            lhsT=ident_r,
                            rhs=sp_sb[t][:, k0:k0 + 512],
                            start=False, stop=True)
                    nc.scalar.activation(
                        P[:, half * 1024:(half + 1) * 1024], L, AF.Exp,
                        accum_out=Zc[:, half:half + 1])
                # 1 / Z
                rz = z_pool.tile([128, 1], f32, tag="rz", name="rz")
                if nkh == 1:
                    nc.vector.reciprocal(rz, Zc[:, 0:1])
                else:
                    nc.vector.tensor_tensor(rz, Zc[:, 0:1], Zc[:, 1:2],
                                            op=OP.add)
                    for half in range(2, nkh):
                        nc.vector.tensor_tensor(rz, rz, Zc[:, half:half + 1],
                                                op=OP.add)
                    nc.vector.reciprocal(rz, rz)
                rz_t.append(rz)
                # normalized attention tile -> HBM
                attn_f = a_pool.tile([128, nk], f32, tag="attn", name="attn")
                if norm_engine == "gpsimd":
                    nc.gpsimd.tensor_scalar_mul(attn_f, P, rz)
                else:
                    nc.vector.tensor_scalar_mul(attn_f, P, rz)
                nc.sync.dma_start(
                    out=dram["attn_o"][h, t * 128:(t + 1) * 128, :],
                    in_=attn_f)
                # transposes of P into PSUM (bf16), then copy to PT
                for g in range(nk128 // 8):
                    T = t_pool.tile([128, 1024], bf16, tag="t", name="t")
                    for i in range(8):
                        kc = g * 8 + i
                        nc.tensor.transpose(
                            T[:, i * 128:(i + 1) * 128],
                            P[:, kc * 128:(kc + 1) * 128],
                            ident_b)
                    dst = PT[:, g * 8:(g + 1) * 8, t * 128:(t + 1) * 128]
                    src = T.rearrange("p (i q) -> p i q", i=8)
                    copy_i += 1
                    if copy_split and copy_i % copy_split == 0:
                        nc.scalar.copy(dst, src)
                    else:
                        nc.vector.tensor_copy(dst, src)
            # attn @ V (unnormalized), Vt-stationary: out [D_K, nql] psum
            av = av_pool.tile([128, nql], f32, tag="av", name="av")
            for kc in range(nk128):
                nc.tensor.matmul(
                    av[0:D_K, :],
                    lhsT=v_sb[:, kc * D_MODEL + h * D_K:
                              kc * D_MODEL + h * D_K + D_K],
                    rhs=PT[:, kc, :],
                    start=(kc == 0), stop=(kc == nk128 - 1))
            avt = avt_pool.tile([D_K, nql], f32, tag="avt", name="avt")
            nc.vector.tensor_copy(avt, av[0:D_K, :])
            # out-projection per qtile, scaled by 1/Z (factored) + accumulate
            for t in range(nqt):
                pp = pp_pool.tile([128, D_MODEL], f32, tag="pp", name="pp")
                nc.tensor.matmul(pp,
                                 lhsT=avt[:, t * 128:(t + 1) * 128],
                                 rhs=wo_stk[:, h * D_MODEL:(h + 1) * D_MODEL],
                                 start=True, stop=True)
                nc.vector.scalar_tensor_tensor(
                    xacc[t], pp, rz_t[t], xacc[t],
                    op0=OP.mult, op1=OP.add)

        # ------------------------------------------------------------------
        # Stage 4: LayerNorm + output
        # ------------------------------------------------------------------
        ln_pool = ctx.enter_context(tc.tile_pool(name="ln_sb", bufs=4))
        for t in range(nqt):
            x = xacc[t]
            mu = ln_pool.tile([128, 1], f32, tag="mu", name="mu")
            nc.vector.tensor_reduce(mu, x, axis=mybir.AxisListType.X,
                                    op=OP.add)
            nc.vector.tensor_scalar_mul(mu, mu, 1.0 / D_MODEL)
            xc = ln_pool.tile([128, D_MODEL], f32, tag="xc", name="xc")
            nc.vector.tensor_scalar(xc, x, mu, None, op0=OP.subtract)
            sq = ln_pool.tile([128, D_MODEL], f32, tag="sq", name="sq")
            nc.vector.tensor_tensor(sq, xc, xc, op=OP.mult)
            vs = ln_pool.tile([128, 1], f32, tag="vs", name="vs")
            nc.vector.tensor_reduce(vs, sq, axis=mybir.AxisListType.X,
                                    op=OP.add)
            sd = ln_pool.tile([128, 1], f32, tag="sd", name="sd")
            nc.scalar.activation(sd, vs, AF.Sqrt, bias=eps_t,
                                 scale=1.0 / D_MODEL)
            rv = ln_pool.tile([128, 1], f32, tag="rv", name="rv")
            nc.vector.reciprocal(rv, sd)
            on = ln_pool.tile([128, D_MODEL], f32, tag="on", name="on")
            nc.vector.tensor_scalar_mul(on, xc, rv)
            fin = ln_pool.tile([128, D_MODEL], f32, tag="fin", name="fin")
            nc.vector.scalar_tensor_tensor(fin, on, 1.0, lng_bc,
                                           op0=OP.bypass, op1=OP.mult)
            nc.vector.tensor_tensor(fin, fin, lnb_bc, op=OP.add)
            nc.sync.dma_start(out=dram["out_o"][t * 128:(t + 1) * 128, :],
                              in_=fin)


# ---------------------------------------------------------------------------
# Host marshalling
# ---------------------------------------------------------------------------

def marshal(inputs, nql=NQ_FULL // N_CORES, nk=NK_FULL, n_cores=N_CORES):
    """Builds the per-core input maps from the full problem inputs."""
    f32 = np.float32

    def g(name):
        return np.asarray(inputs[name], dtype=f32)

    qf, kf, vf = g("query_features"), g("key_features"), g("value_features")
    qp, kp = g("query_positions"), g("key_positions")
    wq, bq = g("wq"), g("bq")
    wk, bk = g("wk"), g("bk")
    wv, bv = g("wv"), g("bv")
    wo, bo = g("wo"), g("bo")
    ws1, ws2 = g("ws1"), g("ws2")
    bs2 = np.float32(np.asarray(inputs["bs2"]))
    ln_g, ln_b = g("ln_g"), g("ln_b")

    sc = np.float32(1.0 / math.sqrt(D_K))
    wq_s, bq_s = wq * sc, bq * sc

    ones_k = np.ones((1, nk), f32)
    kfT_aug = np.ascontiguousarray(np.vstack([kf.T, ones_k]))
    vfT_aug = np.ascontiguousarray(np.vstack([vf.T, ones_k]))

    wq_slot = np.zeros((2, D_MODEL + 1, 128), f32)
    wk_slot = np.zeros((2, D_MODEL + 1, 128), f32)
    for X in range(2):
        for j in range(4):
            h = 4 * X + j
            wq_slot[X, :D_MODEL, 32 * j:32 * j + D_K] = \
                wq_s[:, D_K * h:D_K * (h + 1)]
            wq_slot[X, D_MODEL, 32 * j:32 * j + D_K] = \
                bq_s[D_K * h:D_K * (h + 1)]
            wk_slot[X, :D_MODEL, 32 * j:32 * j + D_K] = \
                wk[:, D_K * h:D_K * (h + 1)]
            wk_slot[X, D_MODEL, 32 * j:32 * j + D_K] = \
                bk[D_K * h:D_K * (h + 1)]

    wv_aug = np.ascontiguousarray(np.vstack([wv, bv[None, :]]))

    wo_stk = np.zeros((D_K, N_HEADS * D_MODEL), f32)
    for h in range(N_HEADS):
        wo_stk[:, h * D_MODEL:(h + 1) * D_MODEL] = \
            wo[D_K * h:D_K * (h + 1), :]

    # center positions (distance-invariant) to tame |x|^2 cancellation
    m = (qp.mean(axis=0) + kp.mean(axis=0)) * 0.5
    qpc, kpc = qp - m, kp - m
    kaug = np.ascontiguousarray(np.vstack([
        -2.0 * kpc.T, np.ones((1, nk), f32),
        (kpc * kpc).sum(-1)[None, :]])).astype(f32)

    C = np.float32((ws2 * np.maximum(ws1, 0.0)).sum())
    sig_scale = np.full((128, 1), C, f32)
    sig_bias = np.full((128, 1), bs2, f32)
    lng_bc = np.ascontiguousarray(np.tile(ln_g[None, :], (128, 1)))
    lnb_bc = np.ascontiguousarray(np.tile(ln_b[None, :], (128, 1)))
    ident_bf16 = np.eye(128, dtype=ml_dtypes.bfloat16)

    blob128 = np.concatenate([sig_scale, sig_bias, lng_bc, lnb_bc],
                             axis=1).astype(f32)
    shared_right65 = np.concatenate(
        [kfT_aug, vfT_aug, wq_slot[0], wq_slot[1], wk_slot[0], wk_slot[1],
         wv_aug], axis=1).astype(f32)

    in_maps = []
    nqt = nql // 128
    for c in range(n_cores):
        qs = slice(c * nql, (c + 1) * nql)
        qf_c, qpc_c = qf[qs], qpc[qs]
        qfT_aug = np.vstack([qf_c.T, np.ones((1, nql), f32)]).astype(f32)
        qaug = np.vstack([
            qpc_c.T, (qpc_c * qpc_c).sum(-1)[None, :],
            np.ones((1, nql), f32)]).astype(f32)
        qf_res = qf_c + bo[None, :]
        qfres128 = np.concatenate(
            [qf_res[t * 128:(t + 1) * 128, :] for t in range(nqt)],
            axis=1).astype(f32)
        m_ = dict(
            blob65=np.ascontiguousarray(
                np.concatenate([qfT_aug, shared_right65], axis=1)),
            blob5=np.ascontiguousarray(
                np.concatenate([qaug, kaug], axis=1)),
            blob128=blob128, wo_stk=wo_stk,
            qfres128=np.ascontiguousarray(qfres128),
            ident_bf16=ident_bf16,
        )
        in_maps.append(m_)
    return in_maps


# ---------------------------------------------------------------------------
# Entry point
# ---------------------------------------------------------------------------

def _get_program():
    key = ("full", NQ_FULL // N_CORES, NK_FULL)
    if key not in _PROG_CACHE:
        _PROG_CACHE[key] = build_program()
    return _PROG_CACHE[key]


def kernel(**inputs):
    from concourse.bass_utils import run_bass_kernel_spmd

    nc = _get_program()
    in_maps = marshal(inputs)
    res = run_bass_kernel_spmd(nc, in_maps, core_ids=list(range(N_CORES)))
    outs = res.results
    attn = np.concatenate([r["attn_o"] for r in outs], axis=1)
    out = np.concatenate([r["out_o"] for r in outs], axis=0)
    return out.astype(np.float32), attn.astype(np.float32)
